# revision 27
# baseline (speedup 1.0000x reference)
"""Trainium2 Bass kernel for nn_EquiConv2d (equirectangular deformable conv).

Key structural facts exploited (derived from the reference geometry):
  * off_y is exactly longitude-invariant, so each (tap k, row h) samples two
    fixed input rows (iy0, iy0+1) with a constant y-fraction.
  * off_x is longitude-invariant up to the 2*pi wrap: sampling along a row is
    a CIRCULAR shift by a constant s0(k,h) plus a constant x-fraction.
  * v2: the x-fraction interpolation is computed on the vector engines with a
    single fused scalar_tensor_tensor op per tap:
        z = (lesser_slice * ratio) + dominant_slice,  ratio <= 1,
    and the max(fr, 1-fr) factor is folded into the stationary weights, so
    each (tap, row) costs ONE matmul [128 contraction, 64 out, 512 free]
    instead of two.  ~9 matmuls/row instead of ~15.
  * Taps with extreme fr (< 1e-4) collapse to a single raw-slice matmul; the
    fp32 seam variants (slot0_useG / slot1_useF) provably only occur for
    such taps (asserted at build time).
  * Two fp32 oddities handled exactly: tap (k=7,h=255) is identically zero
    and tap (k=1,h=1) samples near the antipode with fp32-noise-scattered
    positions -> 3 extra matmul slots with per-column coefficient vectors
    (active only on the cores owning global row 1).

Sharding: 8 cores = 2 batches x 4 bands of 64 output rows.
"""

import math

import numpy as np

# ----------------------------------------------------------------------------
# problem constants
B, C, H, W = 2, 64, 256, 512
O, KH, KW = 64, 3, 3
K = KH * KW
NCORES = 8
NROW = 64            # output rows per core
NSLOT = 12           # max matmul slots per row (excl. specials)
NSPEC = 3            # special (antipode) slots, accumulated into local row 1
RING = 20            # staged row-pair ring slots
PF = 5               # staging prefetch lead (rows) for the G phase
PFF = 8              # earlier lead for the F phase (so G's F-wait is
                     # already satisfied at SP issue time -> no SP stall)
SLOTW = 2048         # F(1024) + G(1024) columns per ring slot
GOFF = 1024
ZW = 514             # interp z tile width (even-aligned slice + 2)
SKIP_TOL = 1e-4      # fr below this -> single raw slot
ZL = 3               # z tiles computed this many rows ahead
MI = 3               # interp taps kept on DVE per row; the rest fold into
                     # two matmul slots each (DVE stt ~620ns vs matmul ~260ns)

_CACHE = {}


# ----------------------------------------------------------------------------
# host-side geometry tables (must replicate reference fp32 semantics exactly)

def _compute_offsets_jax():
    """Bit-exact replica of reference.equi_offsets on jax CPU."""
    import jax
    import jax.numpy as jnp
    cpu = jax.devices("cpu")[0]
    with jax.default_device(cpu):
        dtype = jnp.float32
        pano_H, pano_W, kH, kW = H, W, KH, KW
        Kk = kH * kW
        u = jnp.arange(pano_W, dtype=dtype)
        v = jnp.arange(pano_H, dtype=dtype)
        phi = (u - pano_W / 2.0) / pano_W * (2.0 * math.pi)
        theta = -(v - pano_H / 2.0) / pano_H * math.pi
        cp, sp = jnp.cos(phi), jnp.sin(phi)
        z, one = jnp.zeros_like(cp), jnp.ones_like(cp)
        Ry = jnp.stack([jnp.stack([cp, z, sp], -1),
                        jnp.stack([z, one, z], -1),
                        jnp.stack([-sp, z, cp], -1)], -2)
        ct, st = jnp.cos(theta), jnp.sin(theta)
        zh, oh = jnp.zeros_like(ct), jnp.ones_like(ct)
        Rx = jnp.stack([jnp.stack([oh, zh, zh], -1),
                        jnp.stack([zh, ct, -st], -1),
                        jnp.stack([zh, st, ct], -1)], -2)
        ROT = jnp.einsum('wij,hjk->hwik', Ry, Rx)
        fov_w = kW * (2.0 * math.pi / pano_W)
        focal = (kW / 2.0) / math.tan(fov_w / 2.0)
        hg = (jnp.arange(kH, dtype=dtype)[:, None] + 0.5 - kH / 2.0)
        wg = (jnp.arange(kW, dtype=dtype)[None, :] + 0.5 - kW / 2.0)
        hg = jnp.broadcast_to(hg, (kH, kW)).reshape(Kk)
        wg = jnp.broadcast_to(wg, (kH, kW)).reshape(Kk)
        rays0 = jnp.stack([wg / focal, hg / focal, jnp.ones(Kk, dtype)], 0)
        rays0 = rays0 / jnp.linalg.norm(rays0, axis=0, keepdims=True)
        rays = jnp.einsum('hwik,kn->hwin', ROT, rays0)
        phi2 = jnp.arctan2(rays[..., 0, :], rays[..., 2, :])
        th2 = jnp.arcsin(jnp.clip(rays[..., 1, :], -1.0, 1.0))
        x = pano_W / (2.0 * math.pi) * phi2 + pano_W / 2.0
        y = pano_H / math.pi * th2 + pano_H / 2.0
        off_x = x - (wg[None, None, :] + u[None, :, None])
        off_y = y - (hg[None, None, :] + v[:, None, None])
        return (np.asarray(jnp.transpose(off_y, (2, 0, 1))),
                np.asarray(jnp.transpose(off_x, (2, 0, 1))))


def _build_tap_tables():
    off_y, off_x = _compute_offsets_jax()
    ky = np.repeat(np.arange(KH), KW).astype(np.float32)
    kx = np.tile(np.arange(KW), KH).astype(np.float32)
    base_x = (np.arange(W, dtype=np.float32) - np.float32(1))
    base_y = (np.arange(H, dtype=np.float32) - np.float32(1))
    px = (base_x[None, None, :] + kx[:, None, None] + off_x).astype(np.float32)
    py = (base_y[None, :, None] + ky[:, None, None] + off_y).astype(np.float32)
    pyc = py[:, :, 0]
    assert np.all(py == pyc[:, :, None]), "off_y not longitude-invariant"

    iy0 = np.floor(pyc).astype(np.int64)
    wy1 = (pyc - np.floor(pyc)).astype(np.float64)
    v0 = (iy0 >= 0) & (iy0 < H)
    v1 = (iy0 + 1 >= 0) & (iy0 + 1 < H)
    cy0 = np.where(v0, 1.0 - wy1, 0.0)
    cy1 = np.where(v1, wy1, 0.0)

    Draw = np.mod((px.astype(np.float64) - np.arange(W)[None, None, :]), 512.0)
    ang = Draw / 512.0 * 2 * np.pi
    mean = np.mod(np.angle(np.exp(1j * ang).mean(axis=2)) / (2 * np.pi) * 512.0,
                  512.0)
    resid = np.mod(Draw - mean[:, :, None] + 256.0, 512.0) - 256.0
    D = mean + np.median(resid, axis=2)
    s0 = np.mod(np.floor(D), 512).astype(np.int64)
    frac = D - np.floor(D)

    special = np.zeros((K, H), dtype=bool)
    special[1, 1] = True
    dead = (cy0 == 0.0) & (cy1 == 0.0)

    Ddev = np.abs(np.mod(Draw - D[:, :, None] + 256.0, 512.0) - 256.0)
    dev = Ddev.max(axis=2)
    bad = (dev > 5e-4) & ~special & ~dead
    assert not bad.any(), f"unrepresentable taps: {np.argwhere(bad)}"

    def ref_coefs(p):
        x0 = math.floor(p)
        fr = p - x0
        out = {}
        for ix, wt in ((x0, 1.0 - fr), (x0 + 1, fr)):
            if 0 <= ix < W and wt != 0.0:
                out[ix] = out.get(ix, 0.0) + wt
        return out

    # seam variant selection: decided by the exact fp32 px at the wrap column
    slot0_useG = np.zeros((K, H), dtype=bool)
    slot1_useF = np.zeros((K, H), dtype=bool)
    for k in range(K):
        for h in range(H):
            if special[k, h] or dead[k, h]:
                continue
            s = int(s0[k, h]); fr = frac[k, h]
            if s >= 1:
                w0 = (512 - s) % 512
                rc = ref_coefs(float(px[k, h, w0]))
                slot0_useG[k, h] = (abs(rc.get(0, 0.0))
                                    < abs(rc.get(0, 0.0) - (1 - fr)))
            w1 = (511 - s) % 512
            rc = ref_coefs(float(px[k, h, w1]))
            slot1_useF[k, h] = (abs(rc.get(0, 0.0) - fr)
                                < abs(rc.get(0, 0.0)))

    # special tap (1,1): per-column coefficients on F offsets 255..257
    pxs = px[1, 1, :].astype(np.float64)
    Gam = np.zeros((3, W), dtype=np.float64)
    for w in range(W):
        p = pxs[w]
        x0 = math.floor(p)
        fr = p - x0
        for ix, wt in ((x0, 1.0 - fr), (x0 + 1, fr)):
            if 0 <= ix < W and wt != 0.0:
                found = False
                for jj in range(3):
                    if (255 + jj + w) % 512 == ix % 512:
                        Gam[jj, w] += wt
                        found = True
                        break
                assert found, (w, p, ix)

    return dict(iy0=iy0, cy0=cy0, cy1=cy1, s0=s0, frac=frac,
                slot0_useG=slot0_useG, slot1_useF=slot1_useF,
                special=special, dead=dead, Gam=Gam)


# ----------------------------------------------------------------------------
# uniform SPMD schedule

def _build_schedule(tt):
    blocks = []
    for blk in range(4):
        h0 = blk * NROW
        ev_of, events, first_use = {}, [], []
        need = np.zeros((NROW, K), np.int64)
        for lh in range(NROW):
            for k in range(K):
                r = int(np.clip(tt['iy0'][k, h0 + lh], 0, 255))
                if r not in ev_of:
                    ev_of[r] = len(events)
                    events.append(r)
                    first_use.append(lh)
                need[lh, k] = ev_of[r]
        blocks.append(dict(events=events, first_use=first_use, need=need))

    E = max(len(b['events']) for b in blocks)
    for b in blocks:
        while len(b['events']) < E:
            b['events'].append(b['events'][-1])

    # uniform staged-count before row lh:  tgt(lh) = U[min(lh+PF, NROW-1)]
    U = np.zeros(NROW, np.int64)
    for lh in range(NROW):
        U[lh] = max(int(np.searchsorted(np.asarray(b['first_use']), lh, 'right'))
                    for b in blocks)
    tgt = np.array([U[min(lh + PF, NROW - 1)] for lh in range(NROW)])

    # ring-overwrite feasibility (F phase stages earliest, at lead PFF)
    tgtF = np.array([U[min(lh + PFF, NROW - 1)] for lh in range(NROW)])
    ls = np.full(E, NROW, np.int64)
    for e in range(E):
        hit = np.where(tgtF > e)[0]
        if len(hit):
            ls[e] = hit[0]
    for b in blocks:
        lastuse = {}
        for lh in range(NROW):
            for k in range(K):
                lastuse[int(b['need'][lh, k])] = lh
        for e in range(RING, E):
            prev = e - RING
            if prev in lastuse:
                assert lastuse[prev] < ls[e], \
                    f"RING={RING} too small: ev{e} overwrites ev{prev} " \
                    f"(lastuse {lastuse[prev]}, staged before row {ls[e]})"
    espc = int(blocks[0]['need'][1, 1])
    return blocks, E, tgt, espc


def _build_slots(tt, blocks):
    """Per band: per-row slot descriptors for the v2 schedule."""
    bands = []
    for blk in range(4):
        need = blocks[blk]['need']
        rows = []
        for lh in range(NROW):
            h = blk * NROW + lh
            singles, interps = [], []
            for k in range(K):
                if tt['dead'][k, h] or tt['special'][k, h]:
                    continue
                e = int(need[lh, k])
                s = int(tt['s0'][k, h])
                fr = float(tt['frac'][k, h])
                g0 = bool(tt['slot0_useG'][k, h]) and s >= 1
                f1 = bool(tt['slot1_useF'][k, h])
                v0 = (GOFF + s - 1) if g0 else s
                v1 = (s + 1) if f1 else (GOFF + s)
                if fr < SKIP_TOL:
                    singles.append(dict(kind='single', k=k, e=e, off=v0,
                                        fac=1.0 - fr))
                elif 1.0 - fr < SKIP_TOL:
                    singles.append(dict(kind='single', k=k, e=e, off=v1,
                                        fac=fr))
                else:
                    assert not g0 and not f1, (blk, lh, k)
                    se = s & ~1
                    if fr <= 0.5:
                        lesser, ratio, fac = 'G', fr / (1.0 - fr), 1.0 - fr
                    else:
                        lesser, ratio, fac = 'F', (1.0 - fr) / fr, fr
                    interps.append(dict(kind='interp', k=k, e=e, se=se,
                                        off=s - se, lesser=lesser,
                                        ratio=ratio, fac=fac))
            # measured on HW: the fused stt is the cheapest DVE op (~620ns);
            # tensor_scalar/tensor_tensor/gpsimd variants are all slower.
            # Balance DVE vs PE by keeping MI interps per row as stt and
            # folding the rest back into two matmul slots (fr in lhsT).
            folded = []
            mi = MI + (lh & 1)
            for sl in interps[mi:]:
                k, e = sl['k'], sl['e']
                h2 = blk * NROW + lh
                s = int(tt['s0'][k, h2])
                fr = float(tt['frac'][k, h2])
                folded.append(dict(kind='single', k=k, e=e, off=s,
                                   fac=1.0 - fr))
                folded.append(dict(kind='single', k=k, e=e, off=GOFF + s,
                                   fac=fr))
            slots = singles + folded + interps[:mi]
            assert len(slots) <= NSLOT, (blk, lh, len(slots))
            rows.append(slots)
        bands.append(rows)
    return bands


# ----------------------------------------------------------------------------
# device program

def _emit_section(tc, aps, tiles, blkinfo, slot_rows, j):
    """Emit one per-band section (all-static APs)."""
    import concourse.mybir as mybir
    nc = tc.nc
    f16 = mybir.dt.float16
    f32 = mybir.dt.float32
    buf, coeft, biast, ltst = tiles
    xb, outd, lt = aps['xb'], aps['out'], aps['lt']
    first_use = blkinfo['first_use']
    E_j = len(first_use)

    cum = [int(np.searchsorted(np.asarray(first_use), lh, 'right'))
           for lh in range(NROW)]
    tgt = [cum[min(lh + PF, NROW - 1)] for lh in range(NROW)]
    tgtF = [cum[min(lh + PFF, NROW - 1)] for lh in range(NROW)]

    def stage(e):
        # F dup on the SP hwdge queue; G (row shifted left by one) loads
        # straight from HBM (no F dependency) on the Activation hwdge
        # queue; seam columns via independent gpsimd memsets.
        base = (e % RING) * SLOTW
        src = xb[e].rearrange("p c w -> (p c) w")
        nc.sync.dma_start(buf[:, base:base + W], src)
        nc.sync.dma_start(buf[:, base + W:base + 2 * W], src)
        gsrc = xb[e][:, :, 1:W].rearrange("p c w -> (p c) w")
        nc.scalar.dma_start(buf[:, base + GOFF:base + GOFF + 511], gsrc)
        nc.scalar.dma_start(
            buf[:, base + GOFF + W:base + GOFF + 2 * W - 1], gsrc)
        nc.gpsimd.memset(buf[:, base + GOFF + 511:base + GOFF + 512], 0.0)
        nc.gpsimd.memset(buf[:, base + GOFF + 2 * W - 1:base + GOFF + 2 * W],
                         0.0)

    psp, ltp, zp, outp = tiles_pools[0]

    def emit_z(lz):
        # z tiles via the fused stt on DVE (fastest measured DVE op),
        # emitted ZL rows ahead of the consuming matmuls
        zts = {}
        for i, sl in enumerate(slot_rows[lz]):
            if sl['kind'] != 'interp':
                continue
            base = (sl['e'] % RING) * SLOTW
            fsl = buf[:, base + sl['se']:base + sl['se'] + ZW]
            gsl = buf[:, base + GOFF + sl['se']:base + GOFF + sl['se'] + ZW]
            in0, in1 = (gsl, fsl) if sl['lesser'] == 'G' else (fsl, gsl)
            zt = zp.tile([128, ZW], f16, tag="z")
            nc.vector.scalar_tensor_tensor(zt, in0, float(sl['ratio']), in1,
                                           op0=mybir.AluOpType.mult,
                                           op1=mybir.AluOpType.add)
            zts[i] = zt
        return zts

    staged = zdone = 0
    zrow = {}
    for lh in range(NROW):
        while staged < tgtF[lh]:
            stage(staged)
            staged += 1
        while zdone < min(lh + ZL + 1, NROW):
            zrow[zdone] = emit_z(zdone)
            zdone += 1
        slots = slot_rows[lh]
        zts = zrow.pop(lh)
        ltt = ltp.tile([128, NSLOT * O], f16, tag="ltt")
        nc.sync.dma_start(ltt, lt[lh])
        ps = psp.tile([O, W], f32, tag="ps")

        nmm = len(slots) + (NSPEC if (j == 0 and lh == 1) else 0)
        mi = 0
        for i, sl in enumerate(slots):
            if sl['kind'] == 'single':
                base = (sl['e'] % RING) * SLOTW
                rhs = buf[:, base + sl['off']:base + sl['off'] + W]
            else:
                rhs = zts[i][:, sl['off']:sl['off'] + W]
            nc.tensor.matmul(ps, ltt[:, i * O:(i + 1) * O], rhs,
                             start=(mi == 0), stop=(mi == nmm - 1))
            mi += 1
        if j == 0 and lh == 1:
            sl11 = int(blkinfo['need'][1, 1])
            sbase = (sl11 % RING) * SLOTW
            for jj in range(NSPEC):
                zt = zp.tile([128, W], f16, tag="spz")
                nc.vector.tensor_mul(
                    zt, buf[:, sbase + 255 + jj:sbase + 255 + jj + W],
                    coeft[:, jj * W:(jj + 1) * W])
                nc.tensor.matmul(ps, ltst[:, jj * O:(jj + 1) * O], zt,
                                 start=False, stop=(mi == nmm - 1))
                mi += 1
        ot = outp.tile([O, W], f16, tag="out")
        nc.scalar.activation(ot, ps,
                             mybir.ActivationFunctionType.Identity,
                             bias=biast, scale=1.0)
        # issue on the Activation hwdge queue: follows act in-order, so its
        # wait is satisfied immediately and never stalls the SP queue
        nc.scalar.dma_start(outd[lh], ot)


tiles_pools = [None]


def _emit_kernel(tc, aps, bands, blocks):
    import concourse.mybir as mybir
    nc = tc.nc
    f16 = mybir.dt.float16
    f32 = mybir.dt.float32

    with tc.tile_pool(name="bigp", bufs=1) as bigp, \
         tc.tile_pool(name="ltp", bufs=6) as ltp, \
         tc.tile_pool(name="zp", bufs=24) as zp, \
         tc.tile_pool(name="psp", bufs=8, space="PSUM") as psp, \
         tc.tile_pool(name="outp", bufs=4) as outp:

        buf = bigp.tile([128, RING * SLOTW], f16)
        coeft = bigp.tile([128, NSPEC * W], f16)
        biast = bigp.tile([O, 1], f32)
        ltst = bigp.tile([128, NSPEC * O], f16)

        blkv = nc.values_load(aps['blkid'][0:1, 0:1],
                              min_val=0, max_val=3,
                              skip_runtime_bounds_check=True)

        nc.sync.dma_start(coeft, aps['coefr'])
        nc.sync.dma_start(biast, aps['biasd'])
        nc.sync.dma_start(ltst, aps['lts'])

        tiles = (buf, coeft, biast, ltst)
        tiles_pools[0] = (psp, ltp, zp, outp)
        for j in range(4):
            with tc.If(blkv == j):
                _emit_section(tc, aps, tiles, blocks[j], bands[j], j)


def _get_compiled():
    """Build tables, schedule, and the Bass program once."""
    if 'prog' in _CACHE:
        return _CACHE['prog']
    import concourse.mybir as mybir
    import concourse.tile as tile
    from concourse import bacc

    tt = _build_tap_tables()
    blocks, E, _tgt, _espc = _build_schedule(tt)
    bands = _build_slots(tt, blocks)

    f16 = mybir.dt.float16
    f32 = mybir.dt.float32
    nc = bacc.Bacc("TRN2", target_bir_lowering=False, debug=False,
                   num_devices=NCORES)
    aps = {
        'xb': nc.dram_tensor("xb", [E, 2, C, W], f16,
                             kind="ExternalInput").ap(),
        'lt': nc.dram_tensor("lt", [NROW, 128, NSLOT * O], f16,
                             kind="ExternalInput").ap(),
        'lts': nc.dram_tensor("lts", [128, NSPEC * O], f16,
                              kind="ExternalInput").ap(),
        'blkid': nc.dram_tensor("blkid", [1, 1], mybir.dt.int32,
                                kind="ExternalInput").ap(),
        'coefr': nc.dram_tensor("coefr", [128, NSPEC * W], f16,
                                kind="ExternalInput").ap(),
        'biasd': nc.dram_tensor("biasd", [O, 1], f32,
                                kind="ExternalInput").ap(),
        'out': nc.dram_tensor("out", [NROW, O, W], f16,
                              kind="ExternalOutput").ap(),
    }
    with tile.TileContext(nc) as tc:
        _emit_kernel(tc, aps, bands, blocks)
    nc.finalize()

    _CACHE['prog'] = (nc, tt, blocks, E, bands)
    return _CACHE['prog']


def _core_inputs(x, weight, bias, tt, blocks, E, bands):
    """Assemble per-core in_maps. Core c = batch (c // 4), band (c % 4)."""
    w3 = weight.reshape(O, C, K).astype(np.float64)
    # W2d[p, k, o]: channel-duplicated weights on the contraction axis
    w2d = np.empty((128, K, O), np.float64)
    w2d[:C] = w3.transpose(1, 2, 0)
    w2d[C:] = w3.transpose(1, 2, 0)
    biasd = np.ascontiguousarray(bias.reshape(O, 1).astype(np.float32))

    lts_on = np.zeros((128, NSPEC * O), np.float16)
    for jj in range(NSPEC):
        lts_on[:C, jj * O:(jj + 1) * O] = w2d[:C, 1, :].astype(np.float16)
    lts_off = np.zeros((128, NSPEC * O), np.float16)

    Gam = tt['Gam'].astype(np.float16)
    coef_on = np.ascontiguousarray(
        np.broadcast_to(Gam[:, None, :], (NSPEC, 128, W))
        .transpose(1, 0, 2).reshape(128, NSPEC * W))
    coef_off = np.zeros((128, NSPEC * W), np.float16)

    lt_blk = []
    for blk in range(4):
        ltv = np.zeros((NROW, 128, NSLOT * O), np.float64)
        for lh in range(NROW):
            h = blk * NROW + lh
            for i, sl in enumerate(bands[blk][lh]):
                k = sl['k']
                cy = np.empty(128, np.float64)
                cy[:64] = tt['cy0'][k, h]
                cy[64:] = tt['cy1'][k, h]
                ltv[lh, :, i * O:(i + 1) * O] = \
                    w2d[:, k, :] * cy[:, None] * sl['fac']
        lt_blk.append(np.ascontiguousarray(ltv).astype(np.float16))

    in_maps = []
    for cid in range(NCORES):
        b, blk = cid // 4, cid % 4
        xz = np.concatenate([x[b], np.zeros((C, 1, W), x.dtype)], axis=1)
        xz = xz.astype(np.float16)
        rows = np.asarray(blocks[blk]['events'], np.int64)
        pair_idx = np.stack([rows, rows + 1], axis=1)       # [E, 2]
        xbv = xz[:, pair_idx, :]                            # [C, E, 2, W]
        xbv = np.ascontiguousarray(xbv.transpose(1, 2, 0, 3))  # [E,2,C,W]
        in_maps.append({
            'xb': xbv,
            'lt': lt_blk[blk],
            'lts': lts_on if blk == 0 else lts_off,
            'blkid': np.array([[blk]], np.int32),
            'coefr': coef_on if blk == 0 else coef_off,
            'biasd': biasd,
        })
    return in_maps


def kernel(x, weight, bias):
    from concourse.bass_utils import run_bass_kernel_spmd
    x = np.asarray(x, dtype=np.float32)
    weight = np.asarray(weight, dtype=np.float32)
    bias = np.asarray(bias, dtype=np.float32)

    nc, tt, blocks, E, bands = _get_compiled()
    in_maps = _core_inputs(x, weight, bias, tt, blocks, E, bands)
    res = run_bass_kernel_spmd(nc, in_maps, core_ids=list(range(NCORES)))

    out = np.empty((B, O, H, W), np.float32)
    for cid in range(NCORES):
        b, blk = cid // 4, cid % 4
        oc = res.results[cid]['out']                        # [NROW, O, W]
        out[b, :, blk * NROW:(blk + 1) * NROW, :] = \
            oc.astype(np.float32).transpose(1, 0, 2)
    return out
